# revision 9
# baseline (speedup 1.0000x reference)
"""Trainium2 Bass kernel for a 2-channel diffusion-reaction PDE step (v2).

dUdt = a*lap(U) + U - U^3 - V - k ;  dVdt = b*lap(V) + U - V
a = sigmoid(a_org)*0.01, dx=0.1 so a/dx^2 = sigmoid(a_org) =: c1 (c1v for V).

Strategy (8 cores x 512 rows, channel-pure 128-row windows, zero halo dup):
  * Per core: 4 windows of 128 rows. Inputs: u16 fp16 (cubic path needs
    precision), v8 fp8e4 (V is never cubed; 6% fp8 err is far inside the
    2e-2 rel tolerance). On-chip, u16 is cast to fp8 into a combined
    uv8[128, 2, W] tile (v8 DMAs straight into plane 1).
  * Stencil on TensorE via fp8 DoubleRow matmuls (0.5 cyc/row):
      mm1: pairs (x[n], x[n+2])   weights (c*I, c*I)       -> left+right taps
      mm2: pairs (u[n+1], v[n+1]) weights (tridiag, +-I)   -> vertical taps,
           diagonal (-4c+1 for U / -4c-1 for V) and the cross term
    Halo rows (window top-1/bottom+1) come from a tiny [3, 2, W] tile via a
    K=3 DR matmul; its third partition is a constant-ones row whose weight
    is -k, folding the reaction constant into PSUM for free.
  * Cubic path on DVE fp16 (sq = u*u, t3 = sq*u), evacU on GpSimd:
    out = psum - t3 (tensor_tensor), evacV on ScalarE: copy psum -> fp8.
  * I/O: u16 + v8 in, dU fp16 + dV fp8 out = 12.6 MB/core, ~30us DMA floor.
"""

import numpy as np
import ml_dtypes

import concourse.bass as bass
import concourse.mybir as mybir
from concourse import bacc
from concourse.tile import TileContext
from concourse.bass_utils import run_bass_kernel_spmd

NX, NY = 4096, 4096
NCORES = 8
RPC = NX // NCORES          # 512 rows per core
NW = RPC // 128             # 4 windows per core
W = NY + 2                  # padded width (left/right BC cols)
CT = 512                    # psum bank width (fp32)
HW_ = 2048                  # half-window width (4 psum banks)
WP = 4112                   # padded plane width for uv8 (16B multiple):
                            # avoids sub-granule RMW races between the v8 DMA
                            # (plane 1) and the DVE u-cast (plane 0)
WF = 4104                   # padded width for fp16 tiles (8208B, 16B mult):
                            # unaligned 8196B pool buffers let each window's
                            # u16 DMA RMW-corrupt the previous buffer's tail

f32 = mybir.dt.float32
f16 = mybir.dt.float16
f8 = mybir.dt.float8e4
ALU = mybir.AluOpType
ACTF = mybir.ActivationFunctionType
DRM = mybir.MatmulPerfMode.DoubleRow
E4 = ml_dtypes.float8_e4m3

_BUILD_CACHE = {}


def _build_nc():
    if "nc" in _BUILD_CACHE:
        return _BUILD_CACHE["nc"]

    nc = bacc.Bacc(None, target_bir_lowering=False)

    u16_d = nc.dram_tensor("u16", [NW, 128, W], f16, kind="ExternalInput")
    v8_d = nc.dram_tensor("v8", [NW, 128, W], f8, kind="ExternalInput")
    # halo: per window, [3 parts, 2 planes, W]: part0 = top halo row,
    # part1 = bottom halo row, part2 = ones; plane0 = U, plane1 = V
    # (part2 is ones only in plane0; plane1 part2 = 0)
    h8_d = nc.dram_tensor("h8", [NW, 3, 2, W], f8, kind="ExternalInput")
    # weights: 4 DR stencil mats, partition-major [128, 4, 2, 128]
    # (slot j: mm1U, mm2U, mm1V, mm2V)
    wm_d = nc.dram_tensor("wm", [128, 4, 2, 128], f8, kind="ExternalInput")
    # halo weights: 2 DR mats [3, 2(slot), 2, 128] (haloU, haloV)
    wh_d = nc.dram_tensor("wh", [3, 2, 2, 128], f8, kind="ExternalInput")
    # cubic weights: -I fp16 for the t3 matmul
    wt_d = nc.dram_tensor("wt", [128, 128], f16, kind="ExternalInput")

    ou_d = nc.dram_tensor("ou", [NW, 128, NY], f16, kind="ExternalOutput")
    ov_d = nc.dram_tensor("ov", [NW, 128, NY], f8, kind="ExternalOutput")

    with TileContext(nc) as tc:
        with tc.tile_pool(name="wp", bufs=1) as wp, \
             tc.tile_pool(name="up", bufs=3) as up, \
             tc.tile_pool(name="xp", bufs=3) as xp, \
             tc.tile_pool(name="hp", bufs=3) as hp, \
             tc.tile_pool(name="sqp", bufs=3) as sqp, \
             tc.tile_pool(name="t3p", bufs=3) as t3p, \
             tc.tile_pool(name="oup", bufs=2) as oup, \
             tc.tile_pool(name="ovp", bufs=2) as ovp, \
             tc.tile_pool(name="psp", bufs=2, space="PSUM") as psp:

            wm = wp.tile([128, 4, 2, 128], f8, tag="wm")
            nc.sync.dma_start(out=wm, in_=wm_d[:, :, :, :])
            wh = wp.tile([3, 2, 2, 128], f8, tag="wh")
            nc.sync.dma_start(out=wh, in_=wh_d[:, :, :, :])
            wt = wp.tile([128, 128], f16, tag="wt")
            nc.sync.dma_start(out=wt, in_=wt_d[:, :])

            for t in range(NW):
                u16f = up.tile([128, WF], f16, tag="u16")
                u16 = u16f[:, 0:W]
                nc.sync.dma_start(out=u16, in_=u16_d[t, :, :])
                uv8 = xp.tile([128, 2, WP], f8, tag="uv8")
                nc.sync.dma_start(out=uv8[:, 1, 0:W], in_=v8_d[t, :, :])
                h8f = hp.tile([3, 2, WP], f8, tag="h8")
                h8 = h8f[:, :, 0:W]
                nc.sync.dma_start(out=h8, in_=h8_d[t, :, :, :])

                # u -> fp8 plane 0 (DVE 2x_2p), cubic path on DVE fp16 2x
                nc.vector.tensor_copy(uv8[:, 0, 0:W], u16)
                sqf = sqp.tile([128, WF], f16, tag="sq")
                sq = sqf[:, 0:W]
                nc.vector.tensor_mul(sq, u16, u16)
                t3f = t3p.tile([128, WF], f16, tag="t3")
                t3 = t3f[:, 0:W]
                nc.vector.tensor_mul(t3, sq, u16)

                o16 = oup.tile([128, NY], f16, tag="o16")
                o8 = ovp.tile([128, NY], f8, tag="o8")

                u8flat = uv8[:, 0, 0:W]
                v8flat = uv8[:, 1, 0:W]

                # per half-window (2048 cols), per channel: one 4-bank psum
                # tile; matmuls per 512-col bank (ISA: out N <= 512 fp32);
                # one big Act evac per psum tile.
                def banks(c0):
                    out = []
                    for b in range(HW_ // CT):
                        cb = c0 + CT * b
                        out.append((cb, slice(CT * b, CT * b + CT)))
                    return out

                # weight-major mm order: 4 same-weight mms back-to-back per
                # weight; single fp8->fp16 dtype switch per U psum tile
                for q in range(NY // HW_):
                    c0 = HW_ * q                 # data col offset (0-based)
                    pU = psp.tile([128, HW_], f32, tag="ps", name=f"u{t}_{q}")
                    for cb, sl in banks(c0):
                        rhs_u = bass.AP(u8flat.tensor, u8flat.offset + cb,
                                        [[2 * WP, 128], [2, 2], [1, CT]])
                        nc.tensor.matmul(pU[:, sl], wm[:, 0], rhs_u,
                                         start=True, stop=False, perf_mode=DRM)
                    for cb, sl in banks(c0):
                        nc.tensor.matmul(pU[:, sl], wm[:, 1],
                                         uv8[:, :, cb + 1:cb + 1 + CT],
                                         start=False, stop=False,
                                         perf_mode=DRM)
                    for cb, sl in banks(c0):
                        nc.tensor.matmul(pU[:, sl], wh[:, 0],
                                         h8[:, :, cb + 1:cb + 1 + CT],
                                         start=False, stop=False,
                                         perf_mode=DRM)
                    for cb, sl in banks(c0):
                        nc.tensor.matmul(pU[:, sl], wt,
                                         t3[:, cb + 1:cb + 1 + CT],
                                         start=False, stop=True)
                    # evacU on ScalarE: copy psum -> fp16 (t3 already in)
                    nc.scalar.copy(o16[:, c0:c0 + HW_], pU)

                    pV = psp.tile([128, HW_], f32, tag="ps", name=f"v{t}_{q}")
                    for cb, sl in banks(c0):
                        rhs_v = bass.AP(v8flat.tensor, v8flat.offset + cb,
                                        [[2 * WP, 128], [2, 2], [1, CT]])
                        nc.tensor.matmul(pV[:, sl], wm[:, 2], rhs_v,
                                         start=True, stop=False, perf_mode=DRM)
                    for cb, sl in banks(c0):
                        nc.tensor.matmul(pV[:, sl], wm[:, 3],
                                         uv8[:, :, cb + 1:cb + 1 + CT],
                                         start=False, stop=False,
                                         perf_mode=DRM)
                    for cb, sl in banks(c0):
                        nc.tensor.matmul(pV[:, sl], wh[:, 1],
                                         h8[:, :, cb + 1:cb + 1 + CT],
                                         start=False, stop=True,
                                         perf_mode=DRM)
                    # evacV on ScalarE: psum * 0.5 -> fp8 (halves the <=1ulp
                    # round-away error; host multiplies by 2)
                    nc.scalar.mul(o8[:, c0:c0 + HW_], pV, 0.5)

                nc.scalar.dma_start(out=ou_d[t, :, :], in_=o16)
                nc.sync.dma_start(out=ov_d[t, :, :], in_=o8)

    nc.compile()
    _BUILD_CACHE["nc"] = nc
    return nc


def _sigmoid64(x):
    return 1.0 / (1.0 + np.exp(-np.float64(x)))


def _make_weights(c1, c1v, k):
    """Returns (wm [128,4,2,128] fp8, wh [3,2,2,128] fp8)."""
    wm = np.zeros((4, 128, 2, 128), dtype=np.float32)
    i = np.arange(128)
    # mm1U: (c1*I, c1*I) on (u[n], u[n+2])
    wm[0, i, 0, i] = c1
    wm[0, i, 1, i] = c1
    # mm2U: (tridiag_u, -I) on (u[n+1], v[n+1])
    wm[1, i, 0, i] = -4.0 * c1 + 1.0
    wm[1, i[:-1], 0, i[:-1] + 1] = c1     # row m-1 -> out m  (weight[p, out])
    wm[1, i[1:], 0, i[1:] - 1] = c1       # row m+1 -> out m
    wm[1, i, 1, i] = -1.0
    # mm1V: (c1v*I, c1v*I) on (v[n], v[n+2])
    wm[2, i, 0, i] = c1v
    wm[2, i, 1, i] = c1v
    # mm2V: (+I, tridiag_v) on (u[n+1], v[n+1])
    wm[3, i, 0, i] = 1.0
    wm[3, i, 1, i] = -4.0 * c1v - 1.0
    wm[3, i[:-1], 1, i[:-1] + 1] = c1v
    wm[3, i[1:], 1, i[1:] - 1] = c1v

    wh = np.zeros((2, 3, 2, 128), dtype=np.float32)
    # haloU: part0 (top halo u) -> out 0; part1 (bottom halo u) -> out 127;
    # part2 (ones) -> -k into every U out row
    wh[0, 0, 0, 0] = c1
    wh[0, 1, 0, 127] = c1
    wh[0, 2, 0, :] = -k
    # haloV
    wh[1, 0, 1, 0] = c1v
    wh[1, 1, 1, 127] = c1v
    # to partition-major layouts
    wm_pm = np.ascontiguousarray(wm.transpose(1, 0, 2, 3))   # [128,4,2,128]
    wh_pm = np.ascontiguousarray(wh.transpose(1, 0, 2, 3))   # [3,2,2,128]
    wt = (-np.eye(128, dtype=np.float32)).astype(np.float16)
    return wm_pm.astype(E4), wh_pm.astype(E4), wt


def _make_in_maps(state, bc, a_org, b_org, k_org):
    c1 = np.float32(_sigmoid64(a_org))
    c1v = np.float32(_sigmoid64(b_org))
    k = np.float32(_sigmoid64(k_org) * 0.01)
    wm, wh, wt = _make_weights(c1, c1v, k)

    st = np.asarray(state)[0]                # [2, NX, NY] fp32
    bcn = np.asarray(bc, dtype=np.float32)

    # padded fp16 U grid and fp8 U/V grids, all [NX+2, NY+2]
    gu16 = np.empty((NX + 2, NY + 2), dtype=np.float16)
    gu16[1:-1, 1:-1] = st[0].astype(np.float16)
    gu16[:, 0] = np.float16(bcn[0, 0, 0])
    gu16[:, -1] = np.float16(bcn[0, 0, 1])
    gu16[0, :] = np.float16(bcn[0, 0, 2])
    gu16[-1, :] = np.float16(bcn[0, 0, 3])
    gu8 = gu16.astype(E4)                    # match on-chip cast path
    gv8 = np.empty((NX + 2, NY + 2), dtype=E4)
    gv8[1:-1, 1:-1] = st[1].astype(np.float16).astype(E4)
    gv8[:, 0] = E4(bcn[0, 1, 0])
    gv8[:, -1] = E4(bcn[0, 1, 1])
    gv8[0, :] = E4(bcn[0, 1, 2])
    gv8[-1, :] = E4(bcn[0, 1, 3])

    in_maps = []
    for c in range(NCORES):
        g0 = RPC * c
        u16 = np.empty((NW, 128, W), dtype=np.float16)
        v8 = np.empty((NW, 128, W), dtype=E4)
        h8 = np.zeros((NW, 3, 2, W), dtype=E4)
        for t in range(NW):
            r0 = g0 + 128 * t                # global data row of window top
            u16[t] = gu16[r0 + 1:r0 + 129, :]
            v8[t] = gv8[r0 + 1:r0 + 129, :]
            h8[t, 0, 0] = gu8[r0, :]         # top halo (u)
            h8[t, 1, 0] = gu8[r0 + 129, :]   # bottom halo (u)
            h8[t, 0, 1] = gv8[r0, :]
            h8[t, 1, 1] = gv8[r0 + 129, :]
            h8[t, 2, 0] = E4(1.0)            # ones row (U plane only)
        in_maps.append({"u16": u16, "v8": v8, "h8": h8, "wm": wm, "wh": wh,
                        "wt": wt})
    return in_maps


def _run(in_maps, trace=False, **kwargs):
    nc = _build_nc()
    return run_bass_kernel_spmd(nc, in_maps, list(range(NCORES)),
                                trace=trace, **kwargs)


def kernel(state, bc, a_org, b_org, k_org):
    in_maps = _make_in_maps(state, bc, a_org, b_org, k_org)
    res = _run(in_maps).results
    full = np.empty((1, 2, NX, NY), dtype=np.float32)
    for c in range(NCORES):
        g0 = RPC * c
        ou = res[c]["ou"].astype(np.float32)   # [NW, 128, NY]
        ov = res[c]["ov"].astype(np.float32)
        full[0, 0, g0:g0 + RPC] = ou.reshape(RPC, NY)
        full[0, 1, g0:g0 + RPC] = ov.reshape(RPC, NY) * 2.0
    return full


# revision 10
# speedup vs baseline: 1.0947x; 1.0947x over previous
"""Trainium2 Bass kernel for a 2-channel diffusion-reaction PDE step (v2).

dUdt = a*lap(U) + U - U^3 - V - k ;  dVdt = b*lap(V) + U - V
a = sigmoid(a_org)*0.01, dx=0.1 so a/dx^2 = sigmoid(a_org) =: c1 (c1v for V).

Strategy (8 cores x 512 rows, channel-pure 128-row windows, zero halo dup):
  * Per core: 4 windows of 128 rows. Inputs: u16 fp16 (cubic path needs
    precision), v8 fp8e4 (V is never cubed; 6% fp8 err is far inside the
    2e-2 rel tolerance). On-chip, u16 is cast to fp8 into a combined
    uv8[128, 2, W] tile (v8 DMAs straight into plane 1).
  * Stencil on TensorE via fp8 DoubleRow matmuls (0.5 cyc/row):
      mm1: pairs (x[n], x[n+2])   weights (c*I, c*I)       -> left+right taps
      mm2: pairs (u[n+1], v[n+1]) weights (tridiag, +-I)   -> vertical taps,
           diagonal (-4c+1 for U / -4c-1 for V) and the cross term
    Halo rows (window top-1/bottom+1) come from a tiny [3, 2, W] tile via a
    K=3 DR matmul; its third partition is a constant-ones row whose weight
    is -k, folding the reaction constant into PSUM for free.
  * Cubic path on DVE fp16 (sq = u*u, t3 = sq*u), evacU on GpSimd:
    out = psum - t3 (tensor_tensor), evacV on ScalarE: copy psum -> fp8.
  * I/O: u16 + v8 in, dU fp16 + dV fp8 out = 12.6 MB/core, ~30us DMA floor.
"""

import numpy as np
import ml_dtypes

import concourse.bass as bass
import concourse.mybir as mybir
from concourse import bacc
from concourse.tile import TileContext
from concourse.bass_utils import run_bass_kernel_spmd

NX, NY = 4096, 4096
NCORES = 8
RPC = NX // NCORES          # 512 rows per core
NW = RPC // 128             # 4 windows per core
W = NY + 2                  # padded width (left/right BC cols)
CT = 512                    # psum bank width (fp32)
HW_ = 2048                  # half-window width (4 psum banks)
WP = 4112                   # padded plane width for uv8 (16B multiple):
                            # avoids sub-granule RMW races between the v8 DMA
                            # (plane 1) and the DVE u-cast (plane 0)
WF = 4104                   # padded width for fp16 tiles (8208B, 16B mult):
                            # unaligned 8196B pool buffers let each window's
                            # u16 DMA RMW-corrupt the previous buffer's tail

f32 = mybir.dt.float32
f16 = mybir.dt.float16
f8 = mybir.dt.float8e4
ALU = mybir.AluOpType
ACTF = mybir.ActivationFunctionType
DRM = mybir.MatmulPerfMode.DoubleRow
E4 = ml_dtypes.float8_e4m3

_BUILD_CACHE = {}


def _build_nc():
    if "nc" in _BUILD_CACHE:
        return _BUILD_CACHE["nc"]

    nc = bacc.Bacc(None, target_bir_lowering=False)

    u16_d = nc.dram_tensor("u16", [NW, 128, W], f16, kind="ExternalInput")
    v8_d = nc.dram_tensor("v8", [NW, 128, W], f8, kind="ExternalInput")
    # halo: per window, [3 parts, 2 planes, W]: part0 = top halo row,
    # part1 = bottom halo row, part2 = ones; plane0 = U, plane1 = V
    # (part2 is ones only in plane0; plane1 part2 = 0)
    h8_d = nc.dram_tensor("h8", [NW, 3, 2, W], f8, kind="ExternalInput")
    # weights: 4 DR stencil mats, partition-major [128, 4, 2, 128]
    # (slot j: mm1U, mm2U, mm1V, mm2V)
    wm_d = nc.dram_tensor("wm", [128, 4, 2, 128], f8, kind="ExternalInput")
    # halo weights: 2 DR mats [3, 2(slot), 2, 128] (haloU, haloV)
    wh_d = nc.dram_tensor("wh", [3, 2, 2, 128], f8, kind="ExternalInput")
    # cubic weights: -I fp16 for the t3 matmul
    wt_d = nc.dram_tensor("wt", [128, 128], f16, kind="ExternalInput")

    ou_d = nc.dram_tensor("ou", [NW, 128, NY], f16, kind="ExternalOutput")
    ov_d = nc.dram_tensor("ov", [NW, 128, NY], f8, kind="ExternalOutput")

    with TileContext(nc) as tc:
        with tc.tile_pool(name="wp", bufs=1) as wp, \
             tc.tile_pool(name="up", bufs=2) as up, \
             tc.tile_pool(name="xp", bufs=2) as xp, \
             tc.tile_pool(name="hp", bufs=2) as hp, \
             tc.tile_pool(name="sqp", bufs=2) as sqp, \
             tc.tile_pool(name="t3p", bufs=2) as t3p, \
             tc.tile_pool(name="oup", bufs=2) as oup, \
             tc.tile_pool(name="ovp", bufs=2) as ovp, \
             tc.tile_pool(name="psp", bufs=2, space="PSUM") as psp:

            wm = wp.tile([128, 4, 2, 128], f8, tag="wm")
            nc.sync.dma_start(out=wm, in_=wm_d[:, :, :, :])
            wh = wp.tile([3, 2, 2, 128], f8, tag="wh")
            nc.sync.dma_start(out=wh, in_=wh_d[:, :, :, :])
            wt = wp.tile([128, 128], f16, tag="wt")
            nc.sync.dma_start(out=wt, in_=wt_d[:, :])

            for t in range(NW):
                u16f = up.tile([128, WF], f16, tag="u16")
                u16 = u16f[:, 0:W]
                nc.sync.dma_start(out=u16, in_=u16_d[t, :, :])
                uv8 = xp.tile([128, 2, WP], f8, tag="uv8")
                nc.sync.dma_start(out=uv8[:, 1, 0:W], in_=v8_d[t, :, :])
                h8f = hp.tile([3, 2, WP], f8, tag="h8")
                h8 = h8f[:, :, 0:W]
                nc.sync.dma_start(out=h8, in_=h8_d[t, :, :, :])

                # u -> fp8 plane 0 (DVE 2x_2p), cubic path on DVE fp16 2x
                nc.vector.tensor_copy(uv8[:, 0, 0:W], u16)
                sqf = sqp.tile([128, WF], f16, tag="sq")
                sq = sqf[:, 0:W]
                nc.vector.tensor_mul(sq, u16, u16)
                t3f = t3p.tile([128, WF], f16, tag="t3")
                t3 = t3f[:, 0:W]
                nc.vector.tensor_mul(t3, sq, u16)

                o16 = oup.tile([128, NY], f16, tag="o16")
                o8 = ovp.tile([128, NY], f8, tag="o8")

                u8flat = uv8[:, 0, 0:W]
                v8flat = uv8[:, 1, 0:W]

                # per half-window (2048 cols), per channel: one 4-bank psum
                # tile; matmuls per 512-col bank (ISA: out N <= 512 fp32);
                # one big Act evac per psum tile.
                def banks(c0):
                    out = []
                    for b in range(HW_ // CT):
                        cb = c0 + CT * b
                        out.append((cb, slice(CT * b, CT * b + CT)))
                    return out

                # weight-major mm order: 4 same-weight mms back-to-back per
                # weight; single fp8->fp16 dtype switch per U psum tile
                for q in range(NY // HW_):
                    c0 = HW_ * q                 # data col offset (0-based)
                    pU = psp.tile([128, HW_], f32, tag="ps", name=f"u{t}_{q}")
                    for cb, sl in banks(c0):
                        rhs_u = bass.AP(u8flat.tensor, u8flat.offset + cb,
                                        [[2 * WP, 128], [2, 2], [1, CT]])
                        nc.tensor.matmul(pU[:, sl], wm[:, 0], rhs_u,
                                         start=True, stop=False, perf_mode=DRM)
                    for cb, sl in banks(c0):
                        nc.tensor.matmul(pU[:, sl], wm[:, 1],
                                         uv8[:, :, cb + 1:cb + 1 + CT],
                                         start=False, stop=False,
                                         perf_mode=DRM)
                    for cb, sl in banks(c0):
                        nc.tensor.matmul(pU[:, sl], wh[:, 0],
                                         h8[:, :, cb + 1:cb + 1 + CT],
                                         start=False, stop=False,
                                         perf_mode=DRM)
                    for cb, sl in banks(c0):
                        nc.tensor.matmul(pU[:, sl], wt,
                                         t3[:, cb + 1:cb + 1 + CT],
                                         start=False, stop=True)
                    # evacU on ScalarE: copy psum -> fp16 (t3 already in)
                    nc.scalar.copy(o16[:, c0:c0 + HW_], pU)

                    pV = psp.tile([128, HW_], f32, tag="ps", name=f"v{t}_{q}")
                    for cb, sl in banks(c0):
                        rhs_v = bass.AP(v8flat.tensor, v8flat.offset + cb,
                                        [[2 * WP, 128], [2, 2], [1, CT]])
                        nc.tensor.matmul(pV[:, sl], wm[:, 2], rhs_v,
                                         start=True, stop=False, perf_mode=DRM)
                    for cb, sl in banks(c0):
                        nc.tensor.matmul(pV[:, sl], wm[:, 3],
                                         uv8[:, :, cb + 1:cb + 1 + CT],
                                         start=False, stop=False,
                                         perf_mode=DRM)
                    for cb, sl in banks(c0):
                        nc.tensor.matmul(pV[:, sl], wh[:, 1],
                                         h8[:, :, cb + 1:cb + 1 + CT],
                                         start=False, stop=True,
                                         perf_mode=DRM)
                    # evacV on ScalarE: psum * 0.5 -> fp8 (halves the <=1ulp
                    # round-away error; host multiplies by 2)
                    nc.scalar.mul(o8[:, c0:c0 + HW_], pV, 0.5)

                nc.scalar.dma_start(out=ou_d[t, :, :], in_=o16)
                nc.sync.dma_start(out=ov_d[t, :, :], in_=o8)

    nc.compile()
    _BUILD_CACHE["nc"] = nc
    return nc


def _sigmoid64(x):
    return 1.0 / (1.0 + np.exp(-np.float64(x)))


def _make_weights(c1, c1v, k):
    """Returns (wm [128,4,2,128] fp8, wh [3,2,2,128] fp8)."""
    wm = np.zeros((4, 128, 2, 128), dtype=np.float32)
    i = np.arange(128)
    # mm1U: (c1*I, c1*I) on (u[n], u[n+2])
    wm[0, i, 0, i] = c1
    wm[0, i, 1, i] = c1
    # mm2U: (tridiag_u, -I) on (u[n+1], v[n+1])
    wm[1, i, 0, i] = -4.0 * c1 + 1.0
    wm[1, i[:-1], 0, i[:-1] + 1] = c1     # row m-1 -> out m  (weight[p, out])
    wm[1, i[1:], 0, i[1:] - 1] = c1       # row m+1 -> out m
    wm[1, i, 1, i] = -1.0
    # mm1V: (c1v*I, c1v*I) on (v[n], v[n+2])
    wm[2, i, 0, i] = c1v
    wm[2, i, 1, i] = c1v
    # mm2V: (+I, tridiag_v) on (u[n+1], v[n+1])
    wm[3, i, 0, i] = 1.0
    wm[3, i, 1, i] = -4.0 * c1v - 1.0
    wm[3, i[:-1], 1, i[:-1] + 1] = c1v
    wm[3, i[1:], 1, i[1:] - 1] = c1v

    wh = np.zeros((2, 3, 2, 128), dtype=np.float32)
    # haloU: part0 (top halo u) -> out 0; part1 (bottom halo u) -> out 127;
    # part2 (ones) -> -k into every U out row
    wh[0, 0, 0, 0] = c1
    wh[0, 1, 0, 127] = c1
    wh[0, 2, 0, :] = -k
    # haloV
    wh[1, 0, 1, 0] = c1v
    wh[1, 1, 1, 127] = c1v
    # to partition-major layouts
    wm_pm = np.ascontiguousarray(wm.transpose(1, 0, 2, 3))   # [128,4,2,128]
    wh_pm = np.ascontiguousarray(wh.transpose(1, 0, 2, 3))   # [3,2,2,128]
    wt = (-np.eye(128, dtype=np.float32)).astype(np.float16)
    return wm_pm.astype(E4), wh_pm.astype(E4), wt


def _make_in_maps(state, bc, a_org, b_org, k_org):
    c1 = np.float32(_sigmoid64(a_org))
    c1v = np.float32(_sigmoid64(b_org))
    k = np.float32(_sigmoid64(k_org) * 0.01)
    wm, wh, wt = _make_weights(c1, c1v, k)

    st = np.asarray(state)[0]                # [2, NX, NY] fp32
    bcn = np.asarray(bc, dtype=np.float32)

    # padded fp16 U grid and fp8 U/V grids, all [NX+2, NY+2]
    gu16 = np.empty((NX + 2, NY + 2), dtype=np.float16)
    gu16[1:-1, 1:-1] = st[0].astype(np.float16)
    gu16[:, 0] = np.float16(bcn[0, 0, 0])
    gu16[:, -1] = np.float16(bcn[0, 0, 1])
    gu16[0, :] = np.float16(bcn[0, 0, 2])
    gu16[-1, :] = np.float16(bcn[0, 0, 3])
    gu8 = gu16.astype(E4)                    # match on-chip cast path
    gv8 = np.empty((NX + 2, NY + 2), dtype=E4)
    gv8[1:-1, 1:-1] = st[1].astype(np.float16).astype(E4)
    gv8[:, 0] = E4(bcn[0, 1, 0])
    gv8[:, -1] = E4(bcn[0, 1, 1])
    gv8[0, :] = E4(bcn[0, 1, 2])
    gv8[-1, :] = E4(bcn[0, 1, 3])

    in_maps = []
    for c in range(NCORES):
        g0 = RPC * c
        u16 = np.empty((NW, 128, W), dtype=np.float16)
        v8 = np.empty((NW, 128, W), dtype=E4)
        h8 = np.zeros((NW, 3, 2, W), dtype=E4)
        for t in range(NW):
            r0 = g0 + 128 * t                # global data row of window top
            u16[t] = gu16[r0 + 1:r0 + 129, :]
            v8[t] = gv8[r0 + 1:r0 + 129, :]
            h8[t, 0, 0] = gu8[r0, :]         # top halo (u)
            h8[t, 1, 0] = gu8[r0 + 129, :]   # bottom halo (u)
            h8[t, 0, 1] = gv8[r0, :]
            h8[t, 1, 1] = gv8[r0 + 129, :]
            h8[t, 2, 0] = E4(1.0)            # ones row (U plane only)
        in_maps.append({"u16": u16, "v8": v8, "h8": h8, "wm": wm, "wh": wh,
                        "wt": wt})
    return in_maps


def _run(in_maps, trace=False, **kwargs):
    nc = _build_nc()
    return run_bass_kernel_spmd(nc, in_maps, list(range(NCORES)),
                                trace=trace, **kwargs)


def kernel(state, bc, a_org, b_org, k_org):
    in_maps = _make_in_maps(state, bc, a_org, b_org, k_org)
    res = _run(in_maps).results
    full = np.empty((1, 2, NX, NY), dtype=np.float32)
    for c in range(NCORES):
        g0 = RPC * c
        ou = res[c]["ou"].astype(np.float32)   # [NW, 128, NY]
        ov = res[c]["ov"].astype(np.float32)
        full[0, 0, g0:g0 + RPC] = ou.reshape(RPC, NY)
        full[0, 1, g0:g0 + RPC] = ov.reshape(RPC, NY) * 2.0
    return full
